# revision 19
# baseline (speedup 1.0000x reference)
"""Differential attention on 8 trn2 NeuronCores.

Sharding: data-parallel over batch (2 groups of 4 cores) x tensor-parallel
over heads (4 heads/core). Each core computes its head-group's qkv
projections, dual softmax attention, and a partial output projection over
its 256 channels, plus the per-token sum-of-squares needed for the RMSNorm.
The host sums the 4 partial projections per batch, applies the RMS scale
(which commutes with the channel contraction) and the bias.

v2 architecture: the kernel is Act-engine bound (exp over all N^2 scores at
1 elem/cycle/lane, ~294us/core minimum).  Everything else is scheduled
around keeping the Act engine busy from ~9us onward:
 - All matmul operands are fp16 (same 1 cycle/row PE rate as bf16/f32r,
   half the DMA and SBUF, ~4x better precision than bf16).
 - The QKV/V/output projections are emitted as "filler" work interleaved
   into the attention g-loop, where the PE has ~40% slack, instead of a
   serial up-front phase (which left Act idle ~80us in v1).
 - Score pairs (q1k1 | q2k2) run concurrently in the PE via row tiling
   (both terms are K=64; partition bases 0/64 give tile_position (0,0)
   and (64,0) automatically).
 - attn@V for both terms is a single matmul per key tile: lhsT = [V|ones]
   (65 rows) applies to both u1|u2 column blocks; the ones row yields the
   softmax rowsums riding along in PSUM.
 - PSUM map (8 banks): score ring 2x[128,1024] (4) + o-accum [65,1024]
   (2) + scratch 2x[128,512] (2) shared by qkv/v/proj/ssq tiles.
"""
import sys

sys.path.insert(0, "/opt/trn_rl_repo")

import numpy as np

import concourse.bass as bass
import concourse.mybir as mybir
import concourse.tile as tile
from concourse import bacc, bass_utils
from concourse.bass_interp import get_hw_module

F32 = mybir.dt.float32
F16 = mybir.dt.float16
AF = mybir.ActivationFunctionType
OP = mybir.AluOpType
AX = mybir.AxisListType

B, N, DIM = 2, 2048, 1024
H, HD = 16, 64
HPC = 4          # heads per core
CH = HPC * HD    # channels per core (256)
SCALE = HD ** -0.5
EPS = 1e-5
NT = N // 128    # 16 key tiles
QC = N // 512    # 4 query chunks
CT = DIM // 128  # 8 contraction tiles


def build_program(nc):
    xt = nc.dram_tensor("xt", [DIM, N], F16, kind="ExternalInput").ap()
    # wqk packed f-tile-major: [128, ft, ct, 128]
    wqk = nc.dram_tensor("wqk", [128, 8 * CT * 128], F16, kind="ExternalInput").ap()
    wv = nc.dram_tensor("wv", [128, CT * CH], F16, kind="ExternalInput").ap()
    wp = nc.dram_tensor("wp", [CH, DIM], F16, kind="ExternalInput").ap()
    lam = nc.dram_tensor("lam", [1, 4 * HD], F32, kind="ExternalInput").ap()
    out = nc.dram_tensor("out", [DIM, N], F32, kind="ExternalOutput").ap()
    ssq = nc.dram_tensor("ssq", [1, N], F32, kind="ExternalOutput").ap()

    with tile.TileContext(nc) as tc:
        with (
            nc.allow_low_precision(reason="fp16 matmul operands are intentional"),
            tc.tile_pool(name="persist", bufs=1) as pp,
            tc.tile_pool(name="qkp", bufs=8) as qkpool,
            tc.tile_pool(name="up", bufs=8) as upool,
            tc.tile_pool(name="cpool", bufs=2) as cpool,
            tc.tile_pool(name="rpool", bufs=2) as rpool,
            tc.tile_pool(name="obuf", bufs=3) as obuf,
            tc.tile_pool(name="slots", bufs=3, space="PSUM") as slots,
            tc.tile_pool(name="oacc", bufs=1, space="PSUM") as oaccp,
        ):
            # filler psum tiles borrow ring slots (single 8-bank budget:
            # 3x[128,1024] ring + [65,1024] o-accum)
            def scr_tile(name):
                return slots.tile([128, 1024], F32, tag="slot", name=name)
            # ---- constants / lambda (also warms the exp table early) ----
            ones128 = pp.tile([128, 1], F16, tag="ones128")
            nc.vector.memset(ones128.bitcast(mybir.dt.uint16)[:], 0x3C00)
            ones_row = pp.tile([1, HD], F16, tag="onesrow")
            nc.vector.memset(ones_row.bitcast(mybir.dt.uint16)[:], 0x3C00)
            # PE p-state warmup: ~3.5us of dummy matmuls during the DMA head
            # ramps the tensor clock to 2.4GHz before the real pipeline starts
            warm = pp.tile([128, 512], F16, tag="warm")
            nc.vector.memset(warm[:], 0.0)
            wps = scr_tile("warmps")
            for wi in range(17):
                lo = (wi % 2) * 512
                ro = (wi // 2 % 2) * 32
                nc.tensor.matmul(
                    wps[ro:ro + 1, lo:lo + 512], lhsT=ones128[:], rhs=warm[:],
                    start=True, stop=True,
                )
            lam_sb = pp.tile([1, 4 * HD], F32, tag="lam")

            # ---- persistent tiles ----
            vaug = pp.tile([128, HPC, NT, HD + 1], F16, tag="vaug")
            nc.vector.memset(
                vaug[:, :, :, HD:HD + 1].bitcast(mybir.dt.uint16), 0x3C00
            )
            qk = [qkpool.tile([128, N], F16, tag="qk", name=f"qk{i}") for i in range(8)]
            wqk_sb = pp.tile([128, 8, CT, 128], F16, tag="wqk")
            wv_sb = pp.tile([128, CT, CH], F16, tag="wv")
            wp_sb = pp.tile([128, 2, DIM], F16, tag="wp")
            x_sb = pp.tile([128, CT, N], F16, tag="x")
            o_t = [pp.tile([128, N], F16, tag=f"obig{i}", name=f"obig{i}") for i in range(2)]
            sq = [pp.tile([128, N], F16, tag=f"sqt{i}", name=f"sq{i}") for i in range(2)]
            ssq_sb = pp.tile([1, N], F32, tag="ssqs")

            # ---- DMA order: critical-path first ----
            xt_r = xt.rearrange("(t p) n -> p t n", p=128)
            wqk_r = wqk.rearrange("p (f t c) -> p f t c", f=8, t=CT)

            def dma_x(c):
                nc.sync.dma_start(
                    x_sb[:, :, c * 512:(c + 1) * 512], xt_r[:, :, c * 512:(c + 1) * 512]
                )

            # host packs wqk f-tiles in order [k0,q0,k1,q1,k2,q2,k3,q3]
            # so each head's k+q weights arrive in one contiguous DMA
            FTSLOT = {4: 0, 0: 1, 5: 2, 1: 3, 6: 4, 2: 5, 7: 6, 3: 7}

            def dma_wqk_pair(p):
                nc.sync.dma_start(
                    wqk_sb[:, 2 * p:2 * p + 2, :, :], wqk_r[:, 2 * p:2 * p + 2, :, :]
                )

            dma_x(0)
            dma_wqk_pair(0)
            nc.sync.dma_start(wv_sb.rearrange("p t f -> p (t f)")[:], wv[:])
            nc.sync.dma_start(lam_sb[:], lam[:])
            dma_x(1)
            dma_wqk_pair(1)
            dma_x(2)
            dma_wqk_pair(2)
            dma_x(3)
            dma_wqk_pair(3)
            nc.sync.dma_start(wp_sb[:], wp.rearrange("(t p) o -> p t o", p=128))

            # lambda chain (after the lam DMA is emitted; also warms the
            # Act exp table before the first score exp)
            lprod = pp.tile([1, 2 * HD], F32, tag="lprod")
            nc.vector.tensor_mul(lprod[:, 0:HD], lam_sb[:, 0:HD], lam_sb[:, HD:2 * HD])
            nc.vector.tensor_mul(
                lprod[:, HD:2 * HD], lam_sb[:, 2 * HD:3 * HD], lam_sb[:, 3 * HD:4 * HD]
            )
            lsum = pp.tile([1, 2], F32, tag="lsum")
            nc.vector.reduce_sum(lsum[:, 0:1], lprod[:, 0:HD], axis=AX.X)
            nc.vector.reduce_sum(lsum[:, 1:2], lprod[:, HD:2 * HD], axis=AX.X)
            lexp = pp.tile([1, 2], F32, tag="lexp")
            nc.scalar.activation(lexp[:], lsum[:], AF.Exp)
            negl = pp.tile([1, 1], F32, tag="negl")
            # -lambda_full = exp(sum lq2*lk2) - exp(sum lq1*lk1) - 0.8
            nc.vector.tensor_sub(negl[:], lexp[:, 1:2], lexp[:, 0:1])
            nc.vector.tensor_scalar_add(negl[:], negl[:], -0.8)

            # ---- emit helpers (all scratch PSUM tiles are [128,512]) ----
            qk_live = {}

            def emit_v(nt):
                # V in [token, feature] layout, scattered into vaug
                ps = scr_tile(f"psv{nt}")
                for ct in range(CT):
                    nc.tensor.matmul(
                        ps[:, 0:CH],
                        lhsT=x_sb[:, ct, nt * 128:(nt + 1) * 128],
                        rhs=wv_sb[:, ct, :],
                        start=(ct == 0),
                        stop=(ct == CT - 1),
                    )
                nc.vector.tensor_copy(
                    vaug[:, :, nt, 0:HD],
                    ps[:, 0:CH].rearrange("p (h d) -> p h d", d=HD),
                )

            def emit_qk_half(ft, qc, half):
                # Q/K in [feature, token] layout; two half-chains per (ft,qc)
                if half == 0:
                    ps = scr_tile(f"psqk_{ft}_{qc}")
                    qk_live[(ft, qc)] = ps
                else:
                    ps = qk_live.pop((ft, qc))
                for ct in range(half * 4, half * 4 + 4):
                    nc.tensor.matmul(
                        ps[:, 0:512],
                        lhsT=wqk_sb[:, FTSLOT[ft], ct, :],
                        rhs=x_sb[:, ct, qc * 512:(qc + 1) * 512],
                        start=(ct == 0),
                        stop=(ct == CT - 1),
                    )
                if half == 1:
                    nc.vector.tensor_copy(
                        qk[ft][:, qc * 512:(qc + 1) * 512], ps[:, 0:512]
                    )

            def emit_proj(qc, ot):
                ps = scr_tile(f"psp_{qc}_{ot}")
                for t in range(2):
                    nc.tensor.matmul(
                        ps[:, 0:512],
                        lhsT=wp_sb[:, t, ot * 128:(ot + 1) * 128],
                        rhs=o_t[t][:, qc * 512:(qc + 1) * 512],
                        start=(t == 0),
                        stop=(t == 1),
                    )
                ob = obuf.tile([128, 512], F32, tag="ob", name=f"ob_{qc}_{ot}")
                nc.vector.tensor_copy(ob[:], ps[:, 0:512])
                nc.sync.dma_start(
                    out[ot * 128:(ot + 1) * 128, qc * 512:(qc + 1) * 512], ob[:]
                )

            def emit_sq(qc, t):
                nc.vector.tensor_mul(
                    sq[t][:, qc * 512:(qc + 1) * 512],
                    o_t[t][:, qc * 512:(qc + 1) * 512],
                    o_t[t][:, qc * 512:(qc + 1) * 512],
                )

            def emit_ssq(qc):
                ps = scr_tile(f"ssp_{qc}")
                for t in range(2):
                    nc.tensor.matmul(
                        ps[0:1, 0:512],
                        lhsT=ones128[:],
                        rhs=sq[t][:, qc * 512:(qc + 1) * 512],
                        start=(t == 0),
                        stop=(t == 1),
                    )
                nc.vector.tensor_copy(
                    ssq_sb[:, qc * 512:(qc + 1) * 512], ps[0:1, 0:512]
                )

            # ---- filler schedule ----
            # In-loop entries are popped one per g-slot (before the attn@V of
            # g-2, so V(nt) popped at slot <= nt+1 is emission-safe); extras
            # flush after the g-loop (before the combine).
            def K(h, c, a):
                return lambda: emit_qk_half(4 + h, c, a)

            def Q(h, c, a):
                return lambda: emit_qk_half(h, c, a)

            def V(nt):
                return lambda: emit_v(nt)

            def P(qc, ot):
                return lambda: emit_proj(qc, ot)

            def SQ(qc, t):
                return lambda: emit_sq(qc, t)

            def SSQ(qc):
                return lambda: emit_ssq(qc)

            sched = {
                # (0,0) in-loop slots 0..15 then trailing (flushed before the
                # attn@V pend tail, so V(14)/V(15) land first).  The first
                # iteration carries an unavoidable projection backlog; Act
                # idles ~5us at the (0,0)->(0,1) boundary.
                (0, 0): [K(0, 1, 0), K(0, 1, 1), K(0, 2, 0), K(0, 2, 1),
                         V(4), V(5), V(6), V(7), K(0, 3, 0), K(0, 3, 1),
                         V(8), V(9), V(10), V(11), V(12), V(13),
                         V(14), V(15), Q(1, 0, 0), Q(1, 0, 1),
                         K(1, 0, 0), K(1, 0, 1), K(1, 1, 0), K(1, 1, 1)],
                (0, 1): [K(1, 2, 0), K(1, 2, 1), K(1, 3, 0), K(1, 3, 1),
                         Q(2, 0, 0), Q(2, 0, 1), K(2, 0, 0), K(2, 0, 1),
                         K(2, 1, 0), K(2, 1, 1)],
                (0, 2): [K(2, 2, 0), K(2, 2, 1), K(2, 3, 0), K(2, 3, 1),
                         Q(3, 0, 0), Q(3, 0, 1), K(3, 0, 0), K(3, 0, 1),
                         K(3, 1, 0), K(3, 1, 1)],
                (0, 3): [K(3, 2, 0), K(3, 2, 1), K(3, 3, 0), K(3, 3, 1),
                         Q(0, 1, 0), Q(0, 1, 1), Q(1, 1, 0), Q(1, 1, 1)],
                (1, 0): [Q(2, 1, 0), Q(2, 1, 1), None, None, None, None,
                         P(0, 0), P(0, 1), SQ(0, 0), SQ(0, 1)],
                (1, 1): [P(0, 2), P(0, 3), Q(3, 1, 0), Q(3, 1, 1), SSQ(0)],
                (1, 2): [P(0, 4), P(0, 5), Q(0, 2, 0), Q(0, 2, 1)],
                (1, 3): [P(0, 6), P(0, 7), Q(1, 2, 0), Q(1, 2, 1)],
                (2, 0): [Q(2, 2, 0), Q(2, 2, 1), None, None, None, None,
                         P(1, 0), P(1, 1), SQ(1, 0), SQ(1, 1)],
                (2, 1): [P(1, 2), P(1, 3), Q(3, 2, 0), Q(3, 2, 1), SSQ(1)],
                (2, 2): [P(1, 4), P(1, 5), Q(0, 3, 0), Q(0, 3, 1)],
                (2, 3): [P(1, 6), P(1, 7), Q(1, 3, 0), Q(1, 3, 1)],
                (3, 0): [Q(2, 3, 0), Q(2, 3, 1), None, None, None, None,
                         P(2, 0), P(2, 1), SQ(2, 0), SQ(2, 1)],
                (3, 1): [P(2, 2), P(2, 3), Q(3, 3, 0), Q(3, 3, 1), SSQ(2)],
                (3, 2): [P(2, 4), P(2, 5)],
                (3, 3): [P(2, 6), P(2, 7)],
            }

            # ---- head: minimum work to start the attention pipeline ----
            emit_qk_half(4, 0, 0)
            emit_qk_half(4, 0, 1)   # k(h0) chunk 0
            emit_qk_half(0, 0, 0)
            emit_qk_half(0, 0, 1)   # q(h0) chunk 0
            for nt in range(4):
                emit_v(nt)

            carryB = []

            # ---- main Act-paced loop ----
            def emit_sc(qc, h, g):
                sl = slots.tile([128, 1024], F32, tag="slot",
                                name=f"sl_{qc}_{h}_{g}")
                tq, tk = qk[h], qk[4 + h]
                for term in range(2):
                    rb = term * 64
                    nc.tensor.matmul(
                        sl[:, term * 512:(term + 1) * 512],
                        lhsT=tk[rb:rb + 64, g * 128:(g + 1) * 128],
                        rhs=tq[rb:rb + 64, qc * 512:(qc + 1) * 512],
                        start=True,
                        stop=True,
                    )
                return sl

            iters = [(qc, h) for qc in range(QC) for h in range(HPC)]
            sls = []
            for j, (qc, h) in enumerate(iters):
                if True:
                    fillers = list(sched.get((qc, h), []))
                    o = oaccp.tile([HD + 1, 1024], F32, tag="oacc",
                                   name=f"o_{qc}_{h}")
                    pend = []
                    # prefetch into the NEXT iteration's scores at the end of
                    # this one, so the Act engine's next dependency precedes
                    # the attn@V tail in the PE queue.  Disabled out of (0,0),
                    # whose trailing fillers carry the next iteration's K/Q.
                    # cross-iteration prefetch measured SLOWER (it
                    # starves the shared psum ring the fillers borrow from)
                    pre_ok = False

                    def emit_attnv(g, u, o=o, h=h):
                        # PSUM matmul outputs must stay within one 2KB bank:
                        # two 512-wide matmuls sharing the [V|ones] stationary
                        for j in range(2):
                            nc.tensor.matmul(
                                o[:, j * 512:(j + 1) * 512],
                                lhsT=vaug[:, h, g, :],
                                rhs=u[:, j * 512:(j + 1) * 512],
                                start=(g == 0),
                                stop=(g == NT - 1),
                            )

                    # scores run one slot ahead of the g-loop so the
                    # Act engine's next dependency is always first in the
                    # PE queue
                    if not sls:
                        sls = [emit_sc(qc, h, 0), emit_sc(qc, h, 1)]
                    for g in range(NT):
                        u = upool.tile([128, 1024], F16, tag="u",
                                       name=f"u_{qc}_{h}_{g}")
                        nc.scalar.activation(u[:], sls.pop(0)[:], AF.Exp,
                                             scale=SCALE)
                        if g + 2 < NT:
                            sls.append(emit_sc(qc, h, g + 2))
                        elif pre_ok:
                            nqc, nh = iters[j + 1]
                            sls.append(emit_sc(nqc, nh, g + 2 - NT))
                        pend.append((g, u))
                        if fillers:
                            fi = fillers.pop(0)
                            if fi is not None:
                                fi()
                        if g == 4 and carryB:
                            carryB.pop(0)()
                        if g >= 2:
                            emit_attnv(*pend.pop(0))
                    for f in fillers:
                        if f is not None:
                            f()
                    for ent in pend:
                        emit_attnv(*ent)
                    pend.clear()

                    # ---- combine: out_h = num1/r1 - lambda*num2/r2 ----
                    # one big evacuation frees the PSUM o tile fast; the
                    # rowsum row is then re-staged at partition 0 for the
                    # custom-DVE reciprocal.
                    oe = cpool.tile([HD + 1, 1024], F32, tag="oe",
                                    name=f"oe_{qc}_{h}")
                    nc.vector.tensor_copy(oe[:], o[:])
                    rsum = rpool.tile([1, 1024], F32, tag="rsum",
                                      name=f"rsum_{qc}_{h}")
                    nc.vector.tensor_copy(rsum[:], oe[HD:HD + 1, :])
                    rr = rpool.tile([1, 1024], F32, tag="rr", name=f"rr_{qc}_{h}")
                    nc.vector.reciprocal_approx_fast(out=rr[:], in_=rsum[:])
                    # fold -lambda into the second half; produce fp16 copy
                    # for the PE broadcast matmul (K=1: rep = ones^T @ rr)
                    rr16 = rpool.tile([1, 1024], F16, tag="rr16",
                                      name=f"rr16_{qc}_{h}")
                    nc.vector.tensor_scalar_mul(
                        rr16[:, 512:1024], rr[:, 512:1024], negl
                    )
                    nc.vector.tensor_copy(rr16[:, 0:512], rr[:, 0:512])

                    def combine_b(qc=qc, h=h, oe=oe, rr16=rr16):
                        # runs as a micro-slot of the NEXT iteration so the
                        # PE never blocks on the DVE reciprocal chain
                        rep = scr_tile(f"rep_{qc}_{h}")
                        for j in range(2):
                            nc.tensor.matmul(
                                rep[0:HD, j * 512:(j + 1) * 512],
                                lhsT=ones_row[:],
                                rhs=rr16[:, j * 512:(j + 1) * 512],
                                start=True,
                                stop=True,
                            )
                        m1 = rpool.tile([HD, 512], F32, tag="m1",
                                        name=f"m1_{qc}_{h}")
                        nc.vector.tensor_mul(
                            m1[:], oe[0:HD, 0:512], rep[0:HD, 0:512]
                        )
                        m2 = rpool.tile([HD, 512], F32, tag="m2",
                                        name=f"m2_{qc}_{h}")
                        nc.vector.tensor_mul(
                            m2[:], oe[0:HD, 512:1024], rep[0:HD, 512:1024]
                        )
                        nc.vector.tensor_add(
                            o_t[h // 2][
                                (h % 2) * 64:(h % 2) * 64 + 64,
                                qc * 512:(qc + 1) * 512,
                            ],
                            m1[:],
                            m2[:],
                        )
                    carryB.append(combine_b)

            # ---- tail ----
            # keep the PE p-state at 2.4GHz while the last combine's DVE
            # reciprocal chain runs (the rep matmul in carryB waits on it)
            tps = scr_tile("tailwarm")
            for wi in range(10):
                lo = (wi % 2) * 512
                ro = (wi // 2 % 2) * 32
                nc.tensor.matmul(
                    tps[ro:ro + 1, lo:lo + 512], lhsT=ones128[:], rhs=warm[:],
                    start=True, stop=True,
                )
            for cb in carryB:
                cb()
            carryB.clear()
            for ot in range(8):
                emit_proj(3, ot)
            for t in range(2):
                emit_sq(3, t)
            emit_ssq(3)
            nc.sync.dma_start(ssq[:], ssq_sb[:])
    return nc


_CACHE = {}


def get_nc():
    if "nc" not in _CACHE:
        nc = bacc.Bacc(
            "TRN2", target_bir_lowering=False, debug=False, enable_asserts=False
        )
        build_program(nc)
        nc.compile()
        nc.m = get_hw_module(nc.m)
        _CACHE["nc"] = nc
    return _CACHE["nc"]


def make_in_maps(x, qkv_w, proj_w, lambda_q1, lambda_k1, lambda_q2, lambda_k2):
    x = np.asarray(x, np.float32)
    qkv_w = np.asarray(qkv_w, np.float32)
    proj_w = np.asarray(proj_w, np.float32)
    lamv = np.concatenate(
        [np.asarray(a, np.float32) for a in (lambda_q1, lambda_k1, lambda_q2, lambda_k2)]
    )[None, :]
    in_maps = []
    for core in range(8):
        b, hg = core // 4, core % 4
        h0 = hg * HPC
        rows = []
        for h in range(h0, h0 + HPC):
            rows.append(qkv_w[0 * DIM + h * HD:0 * DIM + (h + 1) * HD])
            rows.append(qkv_w[1 * DIM + h * HD:1 * DIM + (h + 1) * HD])
        for h in range(h0, h0 + HPC):
            rows.append(qkv_w[2 * DIM + h * HD:2 * DIM + (h + 1) * HD])
            rows.append(qkv_w[3 * DIM + h * HD:3 * DIM + (h + 1) * HD])
        wqk_tp = np.concatenate(rows, 0).T  # [DIM, 1024]
        # pack f-tile-major [128, ft, ct, 128], f-tiles permuted to
        # [k0,q0,k1,q1,k2,q2,k3,q3] so each head's k+q is one contiguous DMA
        PERM = [4, 0, 5, 1, 6, 2, 7, 3]
        wqk_np = np.ascontiguousarray(
            wqk_tp.reshape(CT, 128, 8, 128).transpose(1, 2, 0, 3)[:, PERM]
            .reshape(128, -1)
        ).astype(np.float16)
        wv_tp = np.concatenate(
            [qkv_w[4 * DIM + h * HD:4 * DIM + (h + 1) * HD] for h in range(h0, h0 + HPC)],
            0,
        ).T  # [DIM, CH]
        wv_np = np.ascontiguousarray(
            wv_tp.reshape(CT, 128, CH).transpose(1, 0, 2).reshape(128, -1)
        ).astype(np.float16)
        wp_np = np.ascontiguousarray(
            proj_w[:, h0 * HD:(h0 + HPC) * HD].T
        ).astype(np.float16)
        in_maps.append(
            {
                "xt": np.ascontiguousarray(x[b].T).astype(np.float16),
                "wqk": wqk_np,
                "wv": wv_np,
                "wp": wp_np,
                "lam": np.ascontiguousarray(lamv),
            }
        )
    return in_maps


def combine(results, proj_b):
    proj_b = np.asarray(proj_b, np.float32)
    y = np.empty((B, N, DIM), np.float32)
    for b in range(B):
        acc = np.zeros((DIM, N), np.float64)
        sqs = np.zeros(N, np.float64)
        for g in range(4):
            rr = results[b * 4 + g]
            acc += rr["out"].astype(np.float64)
            sqs += rr["ssq"][0].astype(np.float64)
        s = 0.2 / np.sqrt(sqs / DIM + EPS)
        y[b] = (acc.T * s[:, None] + proj_b).astype(np.float32)
    return y


def kernel(x, qkv_w, proj_w, proj_b, lambda_q1, lambda_k1, lambda_q2, lambda_k2):
    nc = get_nc()
    in_maps = make_in_maps(
        x, qkv_w, proj_w, lambda_q1, lambda_k1, lambda_q2, lambda_k2
    )
    res = bass_utils.run_bass_kernel_spmd(nc, in_maps, core_ids=list(range(8)))
    return combine(res.results, proj_b)
